# revision 13
# baseline (speedup 1.0000x reference)
"""MoE routing kernel for TRN2 (8 NeuronCores), Bass/Tile.

Data-parallel over samples with a routing-specialized fully-static PE
schedule. Host computes gating (bit-exact jnp ops), then deals samples to
cores by global distinct-expert-count rank so region q of every core has
the same static size R[q] (NSLOT = sum(R) == ceil(total_distinct/8), i.e.
optimal).

Key FLOP cut vs the 3-matmul form: the reference has NO nonlinearity
between conv1 and conv2 (the only ReLU comes after BN), so the host
pre-folds  A[e] = diag(inv[e]) @ (W2[e] @ W1[e])  (fp64) and
bias_pre[e] = inv*(W2@b1 + b2 - mean) + beta.  Per (sample, expert)
"slot" the device then computes only

    h = relu(A[e] @ x[s] + bias_pre)   (mmA: K=128, 4 MMs -> fp16 SBUF)
    z = W3[e] @ h                      (mmZ: K=256, 4 MMs -> fp16 -> HBM)

i.e. 4096 PE cycles/slot instead of 8192. One slot is shared by every
(gate, t) instance that routes sample s to expert e (z-dedup: ~5.4 of 8
instances distinct -> 44 slots/core instead of 64). The per-(gate,sample)
combine y = tw0*z0 + tw1*z1 + tw.b3 is 0.4% of the FLOPs and pure routing
arithmetic; it runs on the host together with the gating, so the device
program is 100% static.

Schedule notes (from trace analysis): steady-state PE cadence is the
warm roofline (216 ns per N=512 MM, LDWEIGHTS hidden by the 64-deep
reorder window), so the remaining time is head/tail/DMA scheduling:
 - weight panels (A.T | W3.T, 512 fp16 cols/slot) are merged into one
   tensor family and DMA'd in ~4-slot chunks, issued ~5 slots ahead on
   two queues (Sync/GpSimd) so instantaneous HBM demand stays well under
   the ~358 GB/s per-core limit (front-loading them stalled LDWEIGHTS);
 - a short warmup matmul burst keeps PE busy from the framework preamble
   until the first input chunks land (~2.5 us DMA completion latency),
   which also releases the HAM clock gate early;
 - h drains of the first slots and z drains of the last slots are split
   across both Scalar and Vector engines to halve pipeline fill/flush
   latency; out-DMAs alternate Sync/GpSimd queues;
 - mmZ of slot d is scheduled 2 slots late so its h dependency is
   long-satisfied.

The Tile program depends only on the region-size vector R (lru-cached;
inputs are deterministic per problem, so it compiles once)."""
import functools

import numpy as np

E, TOP, C, HD, B, H, W_, NG = 8, 2, 128, 256, 64, 32, 32, 4
P = H * W_            # 1024
NCORES = 8
SPC = B // NCORES     # samples (== regions) per core: 8
EPS = 1e-5
NH = 512              # matmul free-dim chunk (one PSUM bank)
WSC = 512             # panel cols per slot: A.T (256) | W3.T (256)
N_WARM = 10           # warmup matmuls (cover initial DMA wait, warm HAM)
LOOKAHEAD = 9         # slots of DMA lead time (~15 us)
N_HEAD = 3            # slots with split (half-size) drains at the start
N_TAIL = 2            # slots with split drains + dual-queue DMA at the end


def _chunks(total, sizes=(2, 2, 4, 4, 4, 6, 6, 8, 8, 8)):
    """Split `total` slots into DMA chunks, small chunks first."""
    out, i = [], 0
    while total > 0:
        s = min(sizes[min(i, len(sizes) - 1)], total)
        out.append(s)
        total -= s
        i += 1
    return out


XQ_CHUNKS = (1, 1, 2, 2, 2)  # region chunks for x loads


@functools.lru_cache(maxsize=2)
def _build_program(Rkey):
    from concourse import bacc, mybir
    import concourse.tile as tile

    R = list(Rkey)
    NSLOT = sum(R)
    WCH = _chunks(NSLOT)
    f32 = mybir.dt.float32
    f16 = mybir.dt.float16
    MCOLS = 2 * NSLOT
    nc = bacc.Bacc("TRN2", target_bir_lowering=False, debug=False)

    slot_region = []
    for q, r in enumerate(R):
        slot_region += [q] * r
    xq_chunk_of = []  # region -> (chunk idx, local idx)
    for ci_, n in enumerate(XQ_CHUNKS):
        for li in range(n):
            xq_chunk_of.append((ci_, li))
    ws_chunk_of = []  # slot -> (chunk idx, local idx)
    for ci_, n in enumerate(WCH):
        for li in range(n):
            ws_chunk_of.append((ci_, li))

    xq_d = [nc.dram_tensor(f"xq{i}", [C, n * P], f16, kind="ExternalInput")
            for i, n in enumerate(XQ_CHUNKS)]
    wp_d = [nc.dram_tensor(f"wp{i}", [C, n * WSC], f16, kind="ExternalInput")
            for i, n in enumerate(WCH)]
    meta_d = nc.dram_tensor("meta", [C, MCOLS], f32, kind="ExternalInput")
    out_d = nc.dram_tensor("out", [NSLOT, C, P], f16, kind="ExternalOutput")

    AOP = mybir.AluOpType

    with tile.TileContext(nc) as tc:
        with tc.tile_pool(name="per", bufs=1) as per, \
             tc.tile_pool(name="hp", bufs=4) as hpool, \
             tc.tile_pool(name="zbp", bufs=6) as zpool, \
             tc.tile_pool(name="ps", bufs=4, space="PSUM") as pspool:

            # ---- persistent tiles ----
            warm = per.tile([C, NH], f16, tag="warm", name="warm")
            meta = per.tile([C, MCOLS], f32, tag="meta", name="meta")
            xq_t = [per.tile([C, n * P], f16, tag=f"xq{i}", name=f"xqt{i}")
                    for i, n in enumerate(XQ_CHUNKS)]
            wp_t = [per.tile([C, n * WSC], f16, tag=f"wp{i}", name=f"wpt{i}")
                    for i, n in enumerate(WCH)]

            # ---- warmup: PE busy from ~t0 while DMAs land ----
            nc.vector.memset(warm[:], 0.0)
            ps_w = pspool.tile([C, P], f32, tag="ps", name="ps_warm")
            for i in range(N_WARM):
                nc.tensor.matmul(ps_w[:, (i % 2) * NH:(i % 2 + 1) * NH],
                                 warm[:, 0:128], warm[:], start=True,
                                 stop=True)

            # ---- input DMA schedule: just-in-time, need-ordered, spread
            # over the Sync and GpSimd queues so the in-flight byte queue
            # stays small and chunk arrival tracks slot consumption.
            ws_first = {}
            base = 0
            for i, n in enumerate(WCH):
                ws_first[i] = base
                base += n
            xq_first = {}
            base = 0
            for i, n in enumerate(XQ_CHUNKS):
                xq_first[i] = sum(R[:base])
                base += n
            # all input triggers ride the GpSimd (SWDGE) queue — inputs
            # are prefetched ~9 slots ahead so SWDGE's higher latency is
            # irrelevant, and an input trigger is never FIFO-queued
            # behind an out trigger blocked on a drain semaphore.
            # Out-DMAs ride Sync (HWDGE), whose lower completion latency
            # matters at the kernel tail.
            # ...except the chunks needed in the first few slots, which
            # ride Sync (HWDGE, lower latency; its first out trigger only
            # enters the FIFO at iter 2, after these).
            entries = []  # (issue_iter, queue, need_key, kind, idx)
            for i in range(len(WCH)):
                q = "sync" if i < 2 else "gps"
                entries.append((max(0, ws_first[i] - LOOKAHEAD), q,
                                ws_first[i], "wp", i))
            for i in range(len(XQ_CHUNKS)):
                q = "sync" if i == 0 else "gps"
                entries.append((max(0, xq_first[i] - LOOKAHEAD), q,
                                xq_first[i] - 0.5, "xq", i))
            entries.append((0, "sync", 0.75, "meta", 0))
            issue_at = {}  # iter -> [(queue, kind, idx)] in need order
            for it, q, need, kind, i in sorted(entries, key=lambda e: e[2]):
                issue_at.setdefault(it, []).append((q, kind, i))

            def emit_dmas(it):
                for q, kind, i in issue_at.pop(it, ()):
                    eng = nc.sync if q == "sync" else nc.gpsimd
                    if kind == "wp":
                        eng.dma_start(out=wp_t[i][:], in_=wp_d[i][:])
                    elif kind == "xq":
                        eng.dma_start(out=xq_t[i][:], in_=xq_d[i][:])
                    else:
                        eng.dma_start(out=meta[:], in_=meta_d[:])

            # drain-engine balancer: pick engine with least queued time
            eng_load = {"act": 0.0, "dve": 0.0}

            def drain(out_ap, in_ap, bias_ap, relu, cols=P):
                w_act = 1.11 * cols / P
                w_dve = 1.27 * cols / P
                a, v = eng_load["act"], eng_load["dve"]
                if a + w_act <= v + w_dve:
                    nc.scalar.activation(
                        out=out_ap, in_=in_ap,
                        func=(mybir.ActivationFunctionType.Relu if relu else
                              mybir.ActivationFunctionType.Identity),
                        bias=bias_ap, scale=1.0)
                    eng_load["act"] = a + w_act
                else:
                    if relu:
                        nc.vector.tensor_scalar(
                            out=out_ap, in0=in_ap, scalar1=bias_ap,
                            scalar2=0.0, op0=AOP.add, op1=AOP.max)
                    else:
                        nc.vector.tensor_scalar_add(
                            out=out_ap, in0=in_ap, scalar1=bias_ap)
                    eng_load["dve"] = v + w_dve

            hsb = {}   # d -> h sbuf tile

            def emit_mmA(d):
                q = slot_region[d]
                xci, xli = xq_chunk_of[q]
                wci, wli = ws_chunk_of[d]
                xt = xq_t[xci]
                wt = wp_t[wci]
                wb = wli * WSC
                xb = xli * P
                psA = [pspool.tile([C, P], f32, tag="ps", name=f"psA{m}_{d}")
                       for m in range(2)]
                ht = hpool.tile([C, 2 * P], f16, tag="h", name=f"h_{d}")
                for m in range(2):
                    lhs = wt[:, wb + m * 128:wb + (m + 1) * 128]
                    for n in range(2):
                        nc.tensor.matmul(
                            psA[m][:, n * NH:(n + 1) * NH], lhs,
                            xt[:, xb + n * NH:xb + (n + 1) * NH],
                            start=True, stop=True)
                    bias = meta[:, 2 * d + m:2 * d + m + 1]
                    if d < N_HEAD:  # halve latency: one half per engine
                        for n in range(2):
                            drain(ht[:, m * P + n * NH:m * P + (n + 1) * NH],
                                  psA[m][:, n * NH:(n + 1) * NH],
                                  bias, True, cols=NH)
                    else:
                        drain(ht[:, m * P:(m + 1) * P], psA[m][:],
                              bias, True)
                hsb[d] = ht

            def emit_mmZ(d, tail=False):
                wci, wli = ws_chunk_of[d]
                wb = wli * WSC + 256
                wt = wp_t[wci]
                ht = hsb.pop(d)
                psZ = pspool.tile([C, P], f32, tag="ps", name=f"psZ_{d}")
                for k in range(2):
                    lhs = wt[:, wb + k * 128:wb + (k + 1) * 128]
                    for n in range(2):
                        nc.tensor.matmul(
                            psZ[:, n * NH:(n + 1) * NH], lhs,
                            ht[:, k * P + n * NH:k * P + (n + 1) * NH],
                            start=(k == 0), stop=(k == 1))
                zt = zpool.tile([C, P], f16, tag="zb", name=f"z_{d}")
                if tail:  # final slots: cut flush latency with fine
                    # pieces across both drain engines and both queues
                    npc = 4 if d == NSLOT - 1 else 2
                    pw = P // npc
                    for p_ in range(npc):
                        lo, hi = p_ * pw, (p_ + 1) * pw
                        if p_ % 2 == 0:
                            nc.scalar.activation(
                                out=zt[:, lo:hi], in_=psZ[:, lo:hi],
                                func=mybir.ActivationFunctionType.Identity,
                                bias=0.0, scale=1.0)
                        else:
                            nc.vector.tensor_scalar_add(
                                out=zt[:, lo:hi], in0=psZ[:, lo:hi],
                                scalar1=0.0)
                        eng = nc.sync if p_ % 2 == 0 else nc.gpsimd
                        eng.dma_start(out=out_d[d][:, lo:hi],
                                      in_=zt[:, lo:hi])
                else:
                    drain(zt[:], psZ[:], 0.0, False)
                    nc.sync.dma_start(out=out_d[d], in_=zt[:])

            # ---- main loop: iter i runs mmA(i), mmZ(i-2) so every h
            # drain has >1.5us slack before its consumer.
            for i in range(NSLOT + 2):
                emit_dmas(i)
                if i < NSLOT:
                    emit_mmA(i)
                if i >= 2:
                    emit_mmZ(i - 2, tail=(i - 2 >= NSLOT - N_TAIL))

    nc.compile()
    return nc


def _gating(x, gates):
    """Host gating, eager jnp op-for-op as the reference (bit-exact)."""
    import jax
    import jax.numpy as jnp

    xj = jnp.asarray(x)
    gj = jnp.asarray(gates)
    x0 = xj.mean(axis=(2, 3))                      # [B, C]
    tis, tws = [], []
    for i in range(NG):
        probs = jax.nn.softmax(x0 @ gj[i], axis=1)  # [B, E]
        top_p, top_i = jax.lax.top_k(probs, TOP)    # [B, TOP]
        tw = jax.nn.softmax(top_p, axis=1)          # [B, TOP]
        tis.append(np.asarray(top_i))
        tws.append(np.asarray(tw).astype(np.float32))
    return np.stack(tis), np.stack(tws)


def build_in_maps(inputs):
    """Gating, schedule, packed fp16 panels, per-core input maps.

    Returns ((in_maps, schedule), None)."""
    x = np.asarray(inputs["x"], dtype=np.float32)
    gates = np.asarray(inputs["gates"], dtype=np.float32)
    W1 = np.asarray(inputs["W1"], dtype=np.float64)
    b1 = np.asarray(inputs["b1"], dtype=np.float64)
    W2 = np.asarray(inputs["W2"], dtype=np.float64)
    b2 = np.asarray(inputs["b2"], dtype=np.float64)
    bn_gamma = np.asarray(inputs["bn_gamma"], dtype=np.float64)
    bn_beta = np.asarray(inputs["bn_beta"], dtype=np.float64)
    bn_mean = np.asarray(inputs["bn_mean"], dtype=np.float64)
    bn_var = np.asarray(inputs["bn_var"], dtype=np.float64)
    W3 = np.asarray(inputs["W3"], dtype=np.float32)
    b3 = np.asarray(inputs["b3"], dtype=np.float32)

    top_i, tw = _gating(x, gates)  # [NG,B,TOP]

    inv = bn_gamma / np.sqrt(bn_var + np.float64(EPS))   # [E, HD]
    # fold conv1+conv2+BN-scale into one matrix (no nonlinearity between
    # conv1 and conv2); bias_pre is the pre-ReLU additive term
    A = inv[:, :, None] * np.matmul(W2, W1)              # [E, HD, C]
    bias_pre = (inv * (np.einsum('ehg,eg->eh', W2, b1) + b2 - bn_mean)
                + bn_beta)                               # [E, HD]
    A = A.astype(np.float32)
    bias_pre = bias_pre.astype(np.float32)

    # per-sample distinct expert sets (first-appearance order)
    esets = []
    for s in range(B):
        seen = []
        for g in range(NG):
            for t in range(TOP):
                e = int(top_i[g, s, t])
                if e not in seen:
                    seen.append(e)
        esets.append(seen)
    dcount = np.array([len(s) for s in esets])

    # deal samples to cores by global rank: region q of core c gets
    # ranks[SPC*q + c]; R[q] = max demand in that rank row.
    ranks = np.argsort(-dcount, kind="stable")
    Rl = [int(max(dcount[ranks[SPC * q + c]] for c in range(NCORES)))
          for q in range(SPC)]
    Rkey = tuple(Rl)
    NSLOT = sum(Rl)
    WCH = _chunks(NSLOT)
    MCOLS = 2 * NSLOT

    # packed fp16 expert panels: A.T (256 cols) | W3.T as 2 k-chunks
    wpanel = np.empty((E, C, WSC), dtype=np.float16)
    for e in range(E):
        wpanel[e, :, 0:256] = A[e].T               # [C, HD]
        w3t = W3[e].T                              # [HD, C]
        wpanel[e, :, 256:384] = w3t[0:128, :]
        wpanel[e, :, 384:512] = w3t[128:256, :]

    xr = x.reshape(B, C, P)
    in_maps = []
    orders = []      # core -> region -> sample
    slot_maps = []   # core -> {(sample, expert): slot}
    for c in range(NCORES):
        order = [int(ranks[SPC * q + c]) for q in range(SPC)]
        orders.append(order)
        slots = []
        slot_of = {}
        for q in range(SPC):
            s = order[q]
            es = esets[s] + [esets[s][0]] * (Rl[q] - len(esets[s]))
            for r, e in enumerate(es):
                if r < len(esets[s]):
                    slot_of[(s, e)] = len(slots)
                slots.append(e)
        slot_maps.append(slot_of)

        wp = np.empty((C, NSLOT * WSC), dtype=np.float16)
        meta = np.zeros((C, MCOLS), dtype=np.float32)
        for d, e in enumerate(slots):
            wp[:, d * WSC:(d + 1) * WSC] = wpanel[e]
            meta[:, 2 * d + 0] = bias_pre[e, 0:128]
            meta[:, 2 * d + 1] = bias_pre[e, 128:256]

        im = {"meta": meta}
        base = 0
        for i, n in enumerate(XQ_CHUNKS):
            xc = np.empty((C, n * P), dtype=np.float16)
            for li in range(n):
                xc[:, li * P:(li + 1) * P] = xr[order[base + li]]
            im[f"xq{i}"] = xc
            base += n
        base = 0
        for i, n in enumerate(WCH):
            im[f"wp{i}"] = np.ascontiguousarray(
                wp[:, base * WSC:(base + n) * WSC])
            base += n
        in_maps.append(im)
    sched = (orders, slot_maps, Rkey, top_i, tw, b3)
    return (in_maps, sched), None


def combine_outputs(results, sched):
    """Host combine: y[g,s] = tw0*z[s,e0] + tw1*z[s,e1] + tw.b3."""
    orders, slot_maps, _Rkey, top_i, tw, b3 = sched
    core_of = {}
    for c in range(NCORES):
        for s in orders[c]:
            core_of[s] = c
    zs = [np.asarray(r["out"], dtype=np.float32) for r in results]
    outs = []
    for g in range(NG):
        og = np.empty((B, C, P), dtype=np.float32)
        for s in range(B):
            c = core_of[s]
            y = None
            for t in range(TOP):
                e = int(top_i[g, s, t])
                w = float(tw[g, s, t])
                zt = zs[c][slot_maps[c][(s, e)]]
                y = w * zt if y is None else y + w * zt
                if np.any(b3[e]):
                    y = y + w * b3[e][:, None]
            og[s] = y
        outs.append(og.reshape(B, C, H, W_))
    return tuple(outs)


def kernel(x, gates, W1, b1, W2, b2, bn_gamma, bn_beta, bn_mean, bn_var,
           W3, b3):
    from concourse.bass_utils import run_bass_kernel_spmd

    built, fb = build_in_maps({
        "x": x, "gates": gates, "W1": W1, "b1": b1, "W2": W2, "b2": b2,
        "bn_gamma": bn_gamma, "bn_beta": bn_beta, "bn_mean": bn_mean,
        "bn_var": bn_var, "W3": W3, "b3": b3,
    })
    if fb is not None:
        return fb
    in_maps, sched = built
    nc = _build_program(sched[2])
    res = run_bass_kernel_spmd(nc, in_maps, list(range(NCORES)))
    return combine_outputs(res.results, sched)


# revision 17
# speedup vs baseline: 1.0512x; 1.0512x over previous
"""MoE routing kernel for TRN2 (8 NeuronCores), Bass/Tile.

Data-parallel over samples with a routing-specialized fully-static PE
schedule. Host computes gating (bit-exact jnp ops), then deals samples to
cores by global distinct-expert-count rank so region q of every core has
the same static size R[q] (NSLOT = sum(R) == ceil(total_distinct/8), i.e.
optimal).

Key FLOP cut vs the 3-matmul form: the reference has NO nonlinearity
between conv1 and conv2 (the only ReLU comes after BN), so the host
pre-folds  A[e] = diag(inv[e]) @ (W2[e] @ W1[e])  (fp64) and
bias_pre[e] = inv*(W2@b1 + b2 - mean) + beta.  Per (sample, expert)
"slot" the device then computes only

    h = relu(A[e] @ x[s] + bias_pre)   (mmA: K=128, 4 MMs -> fp16 SBUF)
    z = W3[e] @ h                      (mmZ: K=256, 4 MMs -> fp16 -> HBM)

i.e. 4096 PE cycles/slot instead of 8192. One slot is shared by every
(gate, t) instance that routes sample s to expert e (z-dedup: ~5.4 of 8
instances distinct -> 44 slots/core instead of 64). The per-(gate,sample)
combine y = tw0*z0 + tw1*z1 + tw.b3 is 0.4% of the FLOPs and pure routing
arithmetic; it runs on the host together with the gating, so the device
program is 100% static.

Schedule notes (from trace analysis): steady-state PE cadence is the
warm roofline (216 ns per N=512 MM, LDWEIGHTS hidden by the 64-deep
reorder window), so the remaining time is head/tail/DMA scheduling:
 - weight panels (A.T | W3.T, 512 fp16 cols/slot) are merged into one
   tensor family and DMA'd in ~4-slot chunks, issued ~5 slots ahead on
   two queues (Sync/GpSimd) so instantaneous HBM demand stays well under
   the ~358 GB/s per-core limit (front-loading them stalled LDWEIGHTS);
 - a short warmup matmul burst keeps PE busy from the framework preamble
   until the first input chunks land (~2.5 us DMA completion latency),
   which also releases the HAM clock gate early;
 - h drains of the first slots and z drains of the last slots are split
   across both Scalar and Vector engines to halve pipeline fill/flush
   latency; out-DMAs alternate Sync/GpSimd queues;
 - mmZ of slot d is scheduled 2 slots late so its h dependency is
   long-satisfied.

The Tile program depends only on the region-size vector R (lru-cached;
inputs are deterministic per problem, so it compiles once)."""
import functools

import numpy as np

E, TOP, C, HD, B, H, W_, NG = 8, 2, 128, 256, 64, 32, 32, 4
P = H * W_            # 1024
NCORES = 8
SPC = B // NCORES     # samples (== regions) per core: 8
EPS = 1e-5
NH = 512              # matmul free-dim chunk (one PSUM bank)
WSC = 512             # panel cols per slot: A.T (256) | W3.T (256)
N_WARM = 10           # warmup matmuls (cover initial DMA wait, warm HAM)
LOOKAHEAD = 8         # slots of DMA lead (issue slots; actual execution
                      # is throttled by the out-trigger drain-waits that
                      # precede each input trigger in the Sync FIFO)
N_HEAD = 3            # slots with split (half-size) drains at the start
N_TAIL = 2            # slots with split drains + dual-queue DMA at the end


def _chunks(total, sizes=(2, 2, 4, 4, 4, 6, 6, 8, 8, 8)):
    """Split `total` slots into DMA chunks, small chunks first."""
    out, i = [], 0
    while total > 0:
        s = min(sizes[min(i, len(sizes) - 1)], total)
        out.append(s)
        total -= s
        i += 1
    return out


XQ_CHUNKS = (1, 1, 2, 2, 2)  # region chunks for x loads


@functools.lru_cache(maxsize=2)
def _build_program(Rkey):
    from concourse import bacc, mybir
    import concourse.tile as tile

    R = list(Rkey)
    NSLOT = sum(R)
    WCH = _chunks(NSLOT)
    f32 = mybir.dt.float32
    f16 = mybir.dt.float16
    MCOLS = 2 * NSLOT
    nc = bacc.Bacc("TRN2", target_bir_lowering=False, debug=False)

    slot_region = []
    for q, r in enumerate(R):
        slot_region += [q] * r
    xq_chunk_of = []  # region -> (chunk idx, local idx)
    for ci_, n in enumerate(XQ_CHUNKS):
        for li in range(n):
            xq_chunk_of.append((ci_, li))
    ws_chunk_of = []  # slot -> (chunk idx, local idx)
    for ci_, n in enumerate(WCH):
        for li in range(n):
            ws_chunk_of.append((ci_, li))

    xq_d = [nc.dram_tensor(f"xq{i}", [C, n * P], f16, kind="ExternalInput")
            for i, n in enumerate(XQ_CHUNKS)]
    wp_d = [nc.dram_tensor(f"wp{i}", [C, n * WSC], f16, kind="ExternalInput")
            for i, n in enumerate(WCH)]
    meta_d = nc.dram_tensor("meta", [C, MCOLS], f32, kind="ExternalInput")
    out_d = nc.dram_tensor("out", [NSLOT, C, P], f16, kind="ExternalOutput")

    AOP = mybir.AluOpType

    with tile.TileContext(nc) as tc:
        with tc.tile_pool(name="per", bufs=1) as per, \
             tc.tile_pool(name="hp", bufs=4) as hpool, \
             tc.tile_pool(name="zbp", bufs=6) as zpool, \
             tc.tile_pool(name="ps", bufs=4, space="PSUM") as pspool:

            # ---- persistent tiles ----
            warm = per.tile([C, NH], f16, tag="warm", name="warm")
            meta = per.tile([C, MCOLS], f32, tag="meta", name="meta")
            xq_t = [per.tile([C, n * P], f16, tag=f"xq{i}", name=f"xqt{i}")
                    for i, n in enumerate(XQ_CHUNKS)]
            wp_t = [per.tile([C, n * WSC], f16, tag=f"wp{i}", name=f"wpt{i}")
                    for i, n in enumerate(WCH)]

            # ---- warmup: PE busy from ~t0 while DMAs land ----
            nc.vector.memset(warm[:], 0.0)
            # dummy activation: forces the 1.3us ACT_TABLE_LOAD to run
            # now (Scalar is idle), not in front of the first real drain
            # where it would serialize behind the meta DMA wait.
            tbl = per.tile([C, 16], f16, tag="tbl", name="tbl")
            nc.scalar.activation(
                out=tbl[:], in_=warm[:, 0:16],
                func=mybir.ActivationFunctionType.Relu, bias=0.0, scale=1.0)
            ps_w = pspool.tile([C, P], f32, tag="ps", name="ps_warm")
            for i in range(N_WARM):
                nc.tensor.matmul(ps_w[:, (i % 2) * NH:(i % 2 + 1) * NH],
                                 warm[:, 0:128], warm[:], start=True,
                                 stop=True)

            # ---- input DMA schedule: just-in-time, need-ordered, spread
            # over the Sync and GpSimd queues so the in-flight byte queue
            # stays small and chunk arrival tracks slot consumption.
            ws_first = {}
            base = 0
            for i, n in enumerate(WCH):
                ws_first[i] = base
                base += n
            xq_first = {}
            base = 0
            for i, n in enumerate(XQ_CHUNKS):
                xq_first[i] = sum(R[:base])
                base += n
            # Everything rides the Sync (HWDGE) queue. Input triggers
            # issued at iter >= 2 sit FIFO-behind out triggers whose
            # drain-sem waits throttle them to slot rate — exactly the
            # prefetch pacing we want (an engine-idle queue would fire
            # all prefetches immediately and saturate HBM at the head).
            # meta goes absolutely first: it gates every drain bias.
            entries = []  # (issue_iter, queue, need_key, kind, idx)
            for i in range(len(WCH)):
                entries.append((max(0, ws_first[i] - LOOKAHEAD), "sync",
                                ws_first[i], "wp", i))
            for i in range(len(XQ_CHUNKS)):
                entries.append((max(0, xq_first[i] - LOOKAHEAD), "sync",
                                xq_first[i] - 0.5, "xq", i))
            entries.append((0, "sync", -1.0, "meta", 0))
            issue_at = {}  # iter -> [(queue, kind, idx)] in need order
            for it, q, need, kind, i in sorted(entries, key=lambda e: e[2]):
                issue_at.setdefault(it, []).append((q, kind, i))

            def emit_dmas(it):
                for q, kind, i in issue_at.pop(it, ()):
                    eng = nc.sync if q == "sync" else nc.gpsimd
                    if kind == "wp":
                        eng.dma_start(out=wp_t[i][:], in_=wp_d[i][:])
                    elif kind == "xq":
                        eng.dma_start(out=xq_t[i][:], in_=xq_d[i][:])
                    else:
                        eng.dma_start(out=meta[:], in_=meta_d[:])

            # drain-engine balancer: pick engine with least queued time
            eng_load = {"act": 0.0, "dve": 0.0}

            def drain(out_ap, in_ap, bias_ap, relu, cols=P):
                w_act = 1.11 * cols / P
                w_dve = 1.27 * cols / P
                a, v = eng_load["act"], eng_load["dve"]
                if a + w_act <= v + w_dve:
                    nc.scalar.activation(
                        out=out_ap, in_=in_ap,
                        func=(mybir.ActivationFunctionType.Relu if relu else
                              mybir.ActivationFunctionType.Identity),
                        bias=bias_ap, scale=1.0)
                    eng_load["act"] = a + w_act
                else:
                    if relu:
                        nc.vector.tensor_scalar(
                            out=out_ap, in0=in_ap, scalar1=bias_ap,
                            scalar2=0.0, op0=AOP.add, op1=AOP.max)
                    else:
                        nc.vector.tensor_scalar_add(
                            out=out_ap, in0=in_ap, scalar1=bias_ap)
                    eng_load["dve"] = v + w_dve

            hsb = {}   # d -> h sbuf tile

            def emit_mmA(d):
                q = slot_region[d]
                xci, xli = xq_chunk_of[q]
                wci, wli = ws_chunk_of[d]
                xt = xq_t[xci]
                wt = wp_t[wci]
                wb = wli * WSC
                xb = xli * P
                psA = [pspool.tile([C, P], f32, tag="ps", name=f"psA{m}_{d}")
                       for m in range(2)]
                ht = hpool.tile([C, 2 * P], f16, tag="h", name=f"h_{d}")
                for m in range(2):
                    lhs = wt[:, wb + m * 128:wb + (m + 1) * 128]
                    for n in range(2):
                        nc.tensor.matmul(
                            psA[m][:, n * NH:(n + 1) * NH], lhs,
                            xt[:, xb + n * NH:xb + (n + 1) * NH],
                            start=True, stop=True)
                    bias = meta[:, 2 * d + m:2 * d + m + 1]
                    if d < N_HEAD:  # halve latency: one half per engine
                        for n in range(2):
                            drain(ht[:, m * P + n * NH:m * P + (n + 1) * NH],
                                  psA[m][:, n * NH:(n + 1) * NH],
                                  bias, True, cols=NH)
                    else:
                        drain(ht[:, m * P:(m + 1) * P], psA[m][:],
                              bias, True)
                hsb[d] = ht

            def emit_mmZ(d, tail=False):
                wci, wli = ws_chunk_of[d]
                wb = wli * WSC + 256
                wt = wp_t[wci]
                ht = hsb.pop(d)
                psZ = pspool.tile([C, P], f32, tag="ps", name=f"psZ_{d}")
                for k in range(2):
                    lhs = wt[:, wb + k * 128:wb + (k + 1) * 128]
                    for n in range(2):
                        nc.tensor.matmul(
                            psZ[:, n * NH:(n + 1) * NH], lhs,
                            ht[:, k * P + n * NH:k * P + (n + 1) * NH],
                            start=(k == 0), stop=(k == 1))
                zt = zpool.tile([C, P], f16, tag="zb", name=f"z_{d}")
                if tail:  # final slots: cut flush latency with fine
                    # pieces across both drain engines; DMAs ride the
                    # two HWDGE queues (Sync + Scalar) — SWDGE (GpSimd)
                    # completion latency is ~1us worse and binds here.
                    npc = 4 if d == NSLOT - 1 else 2
                    pw = P // npc
                    for p_ in range(npc):
                        lo, hi = p_ * pw, (p_ + 1) * pw
                        if p_ % 2 == 0:
                            nc.scalar.activation(
                                out=zt[:, lo:hi], in_=psZ[:, lo:hi],
                                func=mybir.ActivationFunctionType.Identity,
                                bias=0.0, scale=1.0)
                        else:
                            nc.vector.tensor_scalar_add(
                                out=zt[:, lo:hi], in0=psZ[:, lo:hi],
                                scalar1=0.0)
                        eng = nc.scalar if p_ % 2 == 0 else nc.sync
                        eng.dma_start(out=out_d[d][:, lo:hi],
                                      in_=zt[:, lo:hi])
                else:
                    drain(zt[:], psZ[:], 0.0, False)
                    nc.sync.dma_start(out=out_d[d], in_=zt[:])

            # ---- main loop: iter i runs mmA(i), mmZ(i-2) so every h
            # drain has >1.5us slack before its consumer.
            for i in range(NSLOT + 2):
                emit_dmas(i)
                if i < NSLOT:
                    emit_mmA(i)
                if i >= 2:
                    emit_mmZ(i - 2, tail=(i - 2 >= NSLOT - N_TAIL))

    nc.compile()
    return nc


def _gating(x, gates):
    """Host gating, eager jnp op-for-op as the reference (bit-exact)."""
    import jax
    import jax.numpy as jnp

    xj = jnp.asarray(x)
    gj = jnp.asarray(gates)
    x0 = xj.mean(axis=(2, 3))                      # [B, C]
    tis, tws = [], []
    for i in range(NG):
        probs = jax.nn.softmax(x0 @ gj[i], axis=1)  # [B, E]
        top_p, top_i = jax.lax.top_k(probs, TOP)    # [B, TOP]
        tw = jax.nn.softmax(top_p, axis=1)          # [B, TOP]
        tis.append(np.asarray(top_i))
        tws.append(np.asarray(tw).astype(np.float32))
    return np.stack(tis), np.stack(tws)


def build_in_maps(inputs):
    """Gating, schedule, packed fp16 panels, per-core input maps.

    Returns ((in_maps, schedule), None)."""
    x = np.asarray(inputs["x"], dtype=np.float32)
    gates = np.asarray(inputs["gates"], dtype=np.float32)
    W1 = np.asarray(inputs["W1"], dtype=np.float64)
    b1 = np.asarray(inputs["b1"], dtype=np.float64)
    W2 = np.asarray(inputs["W2"], dtype=np.float64)
    b2 = np.asarray(inputs["b2"], dtype=np.float64)
    bn_gamma = np.asarray(inputs["bn_gamma"], dtype=np.float64)
    bn_beta = np.asarray(inputs["bn_beta"], dtype=np.float64)
    bn_mean = np.asarray(inputs["bn_mean"], dtype=np.float64)
    bn_var = np.asarray(inputs["bn_var"], dtype=np.float64)
    W3 = np.asarray(inputs["W3"], dtype=np.float32)
    b3 = np.asarray(inputs["b3"], dtype=np.float32)

    top_i, tw = _gating(x, gates)  # [NG,B,TOP]

    inv = bn_gamma / np.sqrt(bn_var + np.float64(EPS))   # [E, HD]
    # fold conv1+conv2+BN-scale into one matrix (no nonlinearity between
    # conv1 and conv2); bias_pre is the pre-ReLU additive term
    A = inv[:, :, None] * np.matmul(W2, W1)              # [E, HD, C]
    bias_pre = (inv * (np.einsum('ehg,eg->eh', W2, b1) + b2 - bn_mean)
                + bn_beta)                               # [E, HD]
    A = A.astype(np.float32)
    bias_pre = bias_pre.astype(np.float32)

    # per-sample distinct expert sets (first-appearance order)
    esets = []
    for s in range(B):
        seen = []
        for g in range(NG):
            for t in range(TOP):
                e = int(top_i[g, s, t])
                if e not in seen:
                    seen.append(e)
        esets.append(seen)
    dcount = np.array([len(s) for s in esets])

    # deal samples to cores by global rank: region q of core c gets
    # ranks[SPC*q + c]; R[q] = max demand in that rank row.
    ranks = np.argsort(-dcount, kind="stable")
    Rl = [int(max(dcount[ranks[SPC * q + c]] for c in range(NCORES)))
          for q in range(SPC)]
    Rkey = tuple(Rl)
    NSLOT = sum(Rl)
    WCH = _chunks(NSLOT)
    MCOLS = 2 * NSLOT

    # packed fp16 expert panels: A.T (256 cols) | W3.T as 2 k-chunks
    wpanel = np.empty((E, C, WSC), dtype=np.float16)
    for e in range(E):
        wpanel[e, :, 0:256] = A[e].T               # [C, HD]
        w3t = W3[e].T                              # [HD, C]
        wpanel[e, :, 256:384] = w3t[0:128, :]
        wpanel[e, :, 384:512] = w3t[128:256, :]

    xr = x.reshape(B, C, P)
    in_maps = []
    orders = []      # core -> region -> sample
    slot_maps = []   # core -> {(sample, expert): slot}
    for c in range(NCORES):
        order = [int(ranks[SPC * q + c]) for q in range(SPC)]
        orders.append(order)
        slots = []
        slot_of = {}
        for q in range(SPC):
            s = order[q]
            es = esets[s] + [esets[s][0]] * (Rl[q] - len(esets[s]))
            for r, e in enumerate(es):
                if r < len(esets[s]):
                    slot_of[(s, e)] = len(slots)
                slots.append(e)
        slot_maps.append(slot_of)

        wp = np.empty((C, NSLOT * WSC), dtype=np.float16)
        meta = np.zeros((C, MCOLS), dtype=np.float32)
        for d, e in enumerate(slots):
            wp[:, d * WSC:(d + 1) * WSC] = wpanel[e]
            meta[:, 2 * d + 0] = bias_pre[e, 0:128]
            meta[:, 2 * d + 1] = bias_pre[e, 128:256]

        im = {"meta": meta}
        base = 0
        for i, n in enumerate(XQ_CHUNKS):
            xc = np.empty((C, n * P), dtype=np.float16)
            for li in range(n):
                xc[:, li * P:(li + 1) * P] = xr[order[base + li]]
            im[f"xq{i}"] = xc
            base += n
        base = 0
        for i, n in enumerate(WCH):
            im[f"wp{i}"] = np.ascontiguousarray(
                wp[:, base * WSC:(base + n) * WSC])
            base += n
        in_maps.append(im)
    sched = (orders, slot_maps, Rkey, top_i, tw, b3)
    return (in_maps, sched), None


def combine_outputs(results, sched):
    """Host combine: y[g,s] = tw0*z[s,e0] + tw1*z[s,e1] + tw.b3."""
    orders, slot_maps, _Rkey, top_i, tw, b3 = sched
    core_of = {}
    for c in range(NCORES):
        for s in orders[c]:
            core_of[s] = c
    zs = [np.asarray(r["out"], dtype=np.float32) for r in results]
    outs = []
    for g in range(NG):
        og = np.empty((B, C, P), dtype=np.float32)
        for s in range(B):
            c = core_of[s]
            y = None
            for t in range(TOP):
                e = int(top_i[g, s, t])
                w = float(tw[g, s, t])
                zt = zs[c][slot_maps[c][(s, e)]]
                y = w * zt if y is None else y + w * zt
                if np.any(b3[e]):
                    y = y + w * b3[e][:, None]
            og[s] = y
        outs.append(og.reshape(B, C, H, W_))
    return tuple(outs)


def kernel(x, gates, W1, b1, W2, b2, bn_gamma, bn_beta, bn_mean, bn_var,
           W3, b3):
    from concourse.bass_utils import run_bass_kernel_spmd

    built, fb = build_in_maps({
        "x": x, "gates": gates, "W1": W1, "b1": b1, "W2": W2, "b2": b2,
        "bn_gamma": bn_gamma, "bn_beta": bn_beta, "bn_mean": bn_mean,
        "bn_var": bn_var, "W3": W3, "b3": b3,
    })
    if fb is not None:
        return fb
    in_maps, sched = built
    nc = _build_program(sched[2])
    res = run_bass_kernel_spmd(nc, in_maps, list(range(NCORES)))
    return combine_outputs(res.results, sched)
